# revision 6
# baseline (speedup 1.0000x reference)
"""LocalVariation kernel for Trainium2 (8 NeuronCores, data-parallel over batch).

out[b, k, y, x] = x[b, 0, y, x] - xp[b, 0, y + di, x + dj]   (replicate pad)
for the 24 off-center (di, dj) offsets of a 5x5 window.

Sharding: batch 16 -> 2 images per core. The problem is memory-bound: the
output (402 MB fp32) dominates HBM traffic, so the device computes and
stores in bf16 (norm rel-err ~2.4e-3, well inside the 2e-2 gate) and the
host widens to fp32. That halves store traffic and doubles DVE throughput
(bf16 tensor_tensor runs in 2x perf mode).

Device program per image (host pre-pads to [516, 516] bf16):
  - ONE 2.6-MiB DMA loads T[p, c, i*516 + x] = xpad[128c + p + i, x]
    (the overlapping 5-row window makes rows available per-partition; the
    (i, x) dims merge into one contiguous 5160-B run per (p, c)).
  - ONE DVE tensor_sub per 128-row chunk computes all 25 (i, j) blocks at
    once via a 3-free-dim window access pattern (center block = zeros).
  - ONE store per chunk writes the 24 NON-center blocks to a fully
    contiguous device-layout region out[b, c, p, q', x]: the source uses a
    two-run access pattern [[13W, 2], [1, 12W]] that skips the zero center
    block, so no bytes are wasted and the store stays a single DMA with
    ~12-KiB descriptor runs. Stores round-robin over all three DMA-capable
    pipes (two HWDGE rings + the SWDGE queue) so per-store completion
    latency on one pipe hides behind transfers on the others. The host
    permutes [b, c, p, q', x] -> [b, ch, y, x] — layout fixup only.

Steady-state per-iteration time (8 cores, median of paired For_i slopes):
~90 us, vs ~86 us for the pure HBM roofline of the 30.5 MB/core of
traffic (25.2 MB stores + 5.3 MB overlapped loads at ~363 GB/s/core).
"""

import numpy as np
import ml_dtypes

import concourse.bass as bass
import concourse.bacc as bacc
import concourse.mybir as mybir
import concourse.tile as tile
from concourse.bass_utils import run_bass_kernel_spmd

N_CORES = 8
B_FULL = 16
BPC = B_FULL // N_CORES  # images per core
H = W = 512
KSZ = 5
PAD = 2
NBR = KSZ * KSZ - 1  # 24
HP = H + 2 * PAD  # 516
WP = W + 2 * PAD  # 516
BF16 = mybir.dt.bfloat16
NCH = H // 128  # 4 chunks per image
CBLK = KSZ * WP  # free elems per (chunk) block in T: 2580

_NC_CACHE = {}


def _build_image(nc, tin, tout, x, out, b):
    # One load for the whole image: T[p, c, i*WP + x] = xpad[b, 128c + p + i, x]
    T = tin.tile([128, NCH, CBLK], BF16, name=f"T_{b}", tag="T")
    pstep = T.ap[0][0]
    nc.gpsimd.dma_start(
        out=T[:, :, :],
        in_=bass.AP(x, b * HP * WP, [[WP, 128], [128 * WP, NCH], [1, CBLK]]),
    )

    for c in range(NCH):
        # O[p, 5i+j, x] = center - T[p, c, i, j + x]  (one DVE op, FD=12800)
        O = tout.tile([128, KSZ * KSZ, W], BF16, name=f"O_{b}_{c}", tag="O")
        ostep = O.ap[0][0]
        tbase = T.offset + c * CBLK
        center = bass.AP(
            T.tensor, tbase + PAD * WP + PAD, [[pstep, 128], [0, KSZ], [0, KSZ], [1, W]]
        )
        win = bass.AP(T.tensor, tbase, [[pstep, 128], [WP, KSZ], [1, KSZ], [1, W]])
        o3 = bass.AP(O.tensor, O.offset, [[ostep, 128], [KSZ * W, KSZ], [W, KSZ], [1, W]])
        nc.vector.tensor_sub(o3, center, win)

        # one store of the 24 non-center blocks (two source runs skip q=12),
        # dest fully contiguous; engines round-robin over the three DMA pipes
        gi = b * NCH + c
        eng = (nc.sync, nc.scalar, nc.gpsimd)[gi % 3]
        obase = gi * 128 * NBR * W
        eng.dma_start(
            out=bass.AP(out, obase, [[NBR * W, 128], [12 * W, 2], [1, 12 * W]]),
            in_=bass.AP(O.tensor, O.offset, [[ostep, 128], [13 * W, 2], [1, 12 * W]]),
        )


def build(reps=1, tiny_out=False, loop=False):
    """tiny_out=True: bench variant — full-size stores go to an Internal DRAM
    tensor (same HBM traffic) and only a [128, 512] probe is an ExternalOutput,
    so per-call transfer over the axon tunnel is negligible. loop=True wraps
    the body in a For_i hardware loop (cheap to compile at any rep count)."""
    nc = bacc.Bacc("TRN2", target_bir_lowering=False, debug=False, num_devices=N_CORES)
    x = nc.dram_tensor("x", [BPC, HP, WP], BF16, kind="ExternalInput")
    out_kind = "Internal" if tiny_out else "ExternalOutput"
    out = nc.dram_tensor("out", [BPC, NCH, 128, NBR, W], BF16, kind=out_kind)
    probe = (
        nc.dram_tensor("probe", [128, W], BF16, kind="ExternalOutput") if tiny_out else None
    )
    with tile.TileContext(nc) as tc:
        with (
            tc.tile_pool(name="tin", bufs=2) as tin,
            tc.tile_pool(name="tout", bufs=3) as tout,
        ):
            if loop:
                # staggered_reset: no all-engine barrier at the back edge —
                # iterations overlap like the unrolled-reps form does
                with tc.For_i(0, reps, 1, staggered_reset=True):
                    for b in range(BPC):
                        _build_image(nc, tin, tout, x, out, b)
            else:
                for _ in range(reps):
                    for b in range(BPC):
                        _build_image(nc, tin, tout, x, out, b)
            if probe is not None:
                pt = tin.tile([128, W], BF16, name="pt", tag="pt")
                nc.sync.dma_start(out=pt[:, :], in_=bass.AP(out, 0, [[W, 128], [1, W]]))
                nc.sync.dma_start(out=probe.ap(), in_=pt[:, :])
    nc.compile()
    return nc


def _get_nc():
    if "nc" not in _NC_CACHE:
        _NC_CACHE["nc"] = build()
    return _NC_CACHE["nc"]


def pad_input(x):
    """[16, 1, 512, 512] -> replicate-padded [16, 516, 516], bfloat16."""
    xs = np.asarray(x, dtype=np.float32).reshape(B_FULL, H, W)
    xp = np.pad(xs, ((0, 0), (PAD, PAD), (PAD, PAD)), mode="edge")
    return xp.astype(ml_dtypes.bfloat16)


def run(x, trace=False):
    nc = _get_nc()
    xp = pad_input(x)
    in_maps = [
        {"x": np.ascontiguousarray(xp[BPC * i : BPC * (i + 1)])} for i in range(N_CORES)
    ]
    res = run_bass_kernel_spmd(nc, in_maps, core_ids=list(range(N_CORES)), trace=trace)
    full = np.concatenate(
        [np.asarray(res.results[i]["out"]) for i in range(N_CORES)], axis=0
    )  # [16, NCH, 128, 24, W] bf16; q' order == channel order (center absent)
    out = (
        full.transpose(0, 3, 1, 2, 4).reshape(B_FULL, NBR, H, W).astype(np.float32)
    )
    return out, res


def kernel(x):
    return run(x)[0]


# revision 7
# speedup vs baseline: 1.0151x; 1.0151x over previous
"""LocalVariation kernel for Trainium2 (8 NeuronCores, data-parallel over batch).

out[b, k, y, x] = x[b, 0, y, x] - xp[b, 0, y + di, x + dj]   (replicate pad)
for the 24 off-center (di, dj) offsets of a 5x5 window.

Sharding: batch 16 -> 2 images per core. The problem is memory-bound: the
output (402 MB fp32) dominates HBM traffic, so the device computes and
stores in bf16 (norm rel-err ~2.4e-3, well inside the 2e-2 gate) and the
host widens to fp32. That halves store traffic and doubles DVE throughput
(bf16 tensor_tensor runs in 2x perf mode).

Device program per image (host pre-pads to [516, 516] bf16):
  - ONE 2.6-MiB DMA loads T[p, c, i*516 + x] = xpad[128c + p + i, x]
    (the overlapping 5-row window makes rows available per-partition; the
    (i, x) dims merge into one contiguous 5160-B run per (p, c)).
  - ONE DVE tensor_sub per 128-row chunk computes all 25 (i, j) blocks at
    once via a 3-free-dim window access pattern (center block = zeros).
  - ONE store per chunk writes the 24 NON-center blocks to a fully
    contiguous device-layout region out[b, c, p, q', x]: the source uses a
    two-run access pattern [[13W, 2], [1, 12W]] that skips the zero center
    block, so no bytes are wasted and the store stays a single DMA with
    ~12-KiB descriptor runs. Stores round-robin over all three DMA-capable
    pipes (two HWDGE rings + the SWDGE queue) so per-store completion
    latency on one pipe hides behind transfers on the others. The host
    permutes [b, c, p, q', x] -> [b, ch, y, x] — layout fixup only.

Steady-state per-iteration time (8 cores, median of paired For_i slopes):
~90 us, vs ~86 us for the pure HBM roofline of the 30.5 MB/core of
traffic (25.2 MB stores + 5.3 MB overlapped loads at ~363 GB/s/core).
"""

import numpy as np
import ml_dtypes

import concourse.bass as bass
import concourse.bacc as bacc
import concourse.mybir as mybir
import concourse.tile as tile
from concourse.bass_utils import run_bass_kernel_spmd

N_CORES = 8
B_FULL = 16
BPC = B_FULL // N_CORES  # images per core
H = W = 512
KSZ = 5
PAD = 2
NBR = KSZ * KSZ - 1  # 24
HP = H + 2 * PAD  # 516
WP = W + 2 * PAD  # 516
BF16 = mybir.dt.bfloat16
NCH = H // 128  # 4 chunks per image
CBLK = KSZ * WP  # free elems per (chunk) block in T: 2580

_NC_CACHE = {}


def _build_image(nc, tin, tout, x, out, b):
    # One load for the whole image: T[p, c, i*WP + x] = xpad[b, 128c + p + i, x]
    T = tin.tile([128, NCH, CBLK], BF16, name=f"T_{b}", tag="T")
    pstep = T.ap[0][0]
    nc.gpsimd.dma_start(
        out=T[:, :, :],
        in_=bass.AP(x, b * HP * WP, [[WP, 128], [128 * WP, NCH], [1, CBLK]]),
    )

    for c in range(NCH):
        # O[p, 5i+j, x] = center - T[p, c, i, j + x]  (one DVE op, FD=12800)
        O = tout.tile([128, KSZ * KSZ, W], BF16, name=f"O_{b}_{c}", tag="O")
        ostep = O.ap[0][0]
        tbase = T.offset + c * CBLK
        center = bass.AP(
            T.tensor, tbase + PAD * WP + PAD, [[pstep, 128], [0, KSZ], [0, KSZ], [1, W]]
        )
        win = bass.AP(T.tensor, tbase, [[pstep, 128], [WP, KSZ], [1, KSZ], [1, W]])
        o3 = bass.AP(O.tensor, O.offset, [[ostep, 128], [KSZ * W, KSZ], [W, KSZ], [1, W]])
        nc.vector.tensor_sub(o3, center, win)

        # one store of the 24 non-center blocks (two source runs skip q=12),
        # dest fully contiguous; engines round-robin over the three DMA pipes
        gi = b * NCH + c
        eng = (nc.sync, nc.scalar, nc.gpsimd)[gi % 3]
        obase = gi * 128 * NBR * W
        eng.dma_start(
            out=bass.AP(out, obase, [[NBR * W, 128], [12 * W, 2], [1, 12 * W]]),
            in_=bass.AP(O.tensor, O.offset, [[ostep, 128], [13 * W, 2], [1, 12 * W]]),
        )


def build(reps=1, tiny_out=False, loop=False):
    """tiny_out=True: bench variant — full-size stores go to an Internal DRAM
    tensor (same HBM traffic) and only a [128, 512] probe is an ExternalOutput,
    so per-call transfer over the axon tunnel is negligible. loop=True wraps
    the body in a For_i hardware loop (cheap to compile at any rep count)."""
    nc = bacc.Bacc("TRN2", target_bir_lowering=False, debug=False, num_devices=N_CORES)
    x = nc.dram_tensor("x", [BPC, HP, WP], BF16, kind="ExternalInput")
    out_kind = "Internal" if tiny_out else "ExternalOutput"
    out = nc.dram_tensor("out", [BPC, NCH, 128, NBR, W], BF16, kind=out_kind)
    probe = (
        nc.dram_tensor("probe", [128, W], BF16, kind="ExternalOutput") if tiny_out else None
    )
    with tile.TileContext(nc) as tc:
        with (
            tc.tile_pool(name="tin", bufs=2) as tin,
            tc.tile_pool(name="tout", bufs=2) as tout,
        ):
            if loop:
                # staggered_reset: no all-engine barrier at the back edge —
                # iterations overlap like the unrolled-reps form does
                with tc.For_i(0, reps, 1, staggered_reset=True):
                    for b in range(BPC):
                        _build_image(nc, tin, tout, x, out, b)
            else:
                for _ in range(reps):
                    for b in range(BPC):
                        _build_image(nc, tin, tout, x, out, b)
            if probe is not None:
                pt = tin.tile([128, W], BF16, name="pt", tag="pt")
                nc.sync.dma_start(out=pt[:, :], in_=bass.AP(out, 0, [[W, 128], [1, W]]))
                nc.sync.dma_start(out=probe.ap(), in_=pt[:, :])
    nc.compile()
    return nc


def _get_nc():
    if "nc" not in _NC_CACHE:
        _NC_CACHE["nc"] = build()
    return _NC_CACHE["nc"]


def pad_input(x):
    """[16, 1, 512, 512] -> replicate-padded [16, 516, 516], bfloat16."""
    xs = np.asarray(x, dtype=np.float32).reshape(B_FULL, H, W)
    xp = np.pad(xs, ((0, 0), (PAD, PAD), (PAD, PAD)), mode="edge")
    return xp.astype(ml_dtypes.bfloat16)


def run(x, trace=False):
    nc = _get_nc()
    xp = pad_input(x)
    in_maps = [
        {"x": np.ascontiguousarray(xp[BPC * i : BPC * (i + 1)])} for i in range(N_CORES)
    ]
    res = run_bass_kernel_spmd(nc, in_maps, core_ids=list(range(N_CORES)), trace=trace)
    full = np.concatenate(
        [np.asarray(res.results[i]["out"]) for i in range(N_CORES)], axis=0
    )  # [16, NCH, 128, 24, W] bf16; q' order == channel order (center absent)
    out = (
        full.transpose(0, 3, 1, 2, 4).reshape(B_FULL, NBR, H, W).astype(np.float32)
    )
    return out, res


def kernel(x):
    return run(x)[0]


# revision 8
# speedup vs baseline: 1.0983x; 1.0820x over previous
"""LocalVariation kernel for Trainium2 (8 NeuronCores, data-parallel over batch).

out[b, k, y, x] = x[b, 0, y, x] - xp[b, 0, y + di, x + dj]   (replicate pad)
for the 24 off-center (di, dj) offsets of a 5x5 window.

Sharding: batch 16 -> 2 images per core. The problem is memory-bound: the
output (402 MB fp32) dominates HBM traffic, so the device computes and
stores in bf16 (norm rel-err ~2.4e-3, well inside the 2e-2 gate) and the
host widens to fp32.

Band tiling: partition p holds rows 4p .. 4p+7 of the padded image (4
output rows + 4 halo rows, ONE contiguous 8.25-KiB DMA run per
partition), so the whole-image load is 1.06 MB -- a 2x halo overlap
instead of the 5x of chunk-per-128-rows window tiling. Per image:
  - ONE 1.06-MB load of the band tile T[p, 0:8*516].
  - FOUR DVE tensor_subs (r = 0..3), each computing all 25 (i, j) blocks
    for output rows 4p + r via a 3-free-dim window access pattern
    (bf16 2x perf mode; center block = zeros).
  - FOUR stores, one per r, each writing the 24 NON-center blocks to a
    fully contiguous device-layout region out[b, r, p, q', x]: the source
    two-run access pattern [[13W, 2], [1, 12W]] skips the zero center
    block. Stores round-robin over all three DMA-capable pipes (two HWDGE
    rings + the SWDGE queue) so per-store completion latency hides behind
    the other pipes. The host permutes [b, r, p, q', x] -> [b, ch, 4p+r,
    x] -- layout fixup only, no compute.

Steady-state per-iteration time (8 cores, median of paired For_i slopes):
~84-88 us, vs ~75 us for the pure HBM roofline of the 27.3 MB/core of
traffic (25.2 MB stores + 2.1 MB loads at ~363 GB/s/core).
"""

import numpy as np
import ml_dtypes

import concourse.bass as bass
import concourse.bacc as bacc
import concourse.mybir as mybir
import concourse.tile as tile
from concourse.bass_utils import run_bass_kernel_spmd

N_CORES = 8
B_FULL = 16
BPC = B_FULL // N_CORES  # images per core
H = W = 512
KSZ = 5
PAD = 2
NBR = KSZ * KSZ - 1  # 24
HP = H + 2 * PAD  # 516
WP = W + 2 * PAD  # 516
BF16 = mybir.dt.bfloat16
NR = 4  # output rows per partition (128 * 4 = 512)
BAND = (NR + KSZ - 1) * WP  # elems per partition band: 8 rows

_NC_CACHE = {}


def _build_image(nc, tin, tout, x, out, b):
    # One load per image: T[p, k] = xpad[b, 4p + k // WP, k % WP]  (8 rows/part)
    T = tin.tile([128, BAND], BF16, name=f"T_{b}", tag="T")
    pstep = T.ap[0][0]
    nc.gpsimd.dma_start(
        out=T[:, :],
        in_=bass.AP(x, b * HP * WP, [[NR * WP, 128], [1, BAND]]),
    )

    for r in range(NR):
        # O[p, 5i+j, x] = center - window  for output row 4p + r (FD=12800)
        O = tout.tile([128, KSZ * KSZ, W], BF16, name=f"O_{b}_{r}", tag="O")
        ostep = O.ap[0][0]
        tbase = T.offset + r * WP
        center = bass.AP(
            T.tensor, tbase + PAD * WP + PAD, [[pstep, 128], [0, KSZ], [0, KSZ], [1, W]]
        )
        win = bass.AP(T.tensor, tbase, [[pstep, 128], [WP, KSZ], [1, KSZ], [1, W]])
        o3 = bass.AP(O.tensor, O.offset, [[ostep, 128], [KSZ * W, KSZ], [W, KSZ], [1, W]])
        nc.vector.tensor_sub(o3, center, win)

        # one store of the 24 non-center blocks (two source runs skip q=12),
        # dest fully contiguous; engines round-robin over the three DMA pipes
        gi = b * NR + r
        eng = (nc.sync, nc.scalar, nc.gpsimd)[gi % 3]
        obase = gi * 128 * NBR * W
        eng.dma_start(
            out=bass.AP(out, obase, [[NBR * W, 128], [12 * W, 2], [1, 12 * W]]),
            in_=bass.AP(O.tensor, O.offset, [[ostep, 128], [13 * W, 2], [1, 12 * W]]),
        )


def build(reps=1, tiny_out=False, loop=False):
    """tiny_out=True: bench variant — full-size stores go to an Internal DRAM
    tensor (same HBM traffic) and only a [128, 512] probe is an ExternalOutput,
    so per-call transfer over the axon tunnel is negligible. loop=True wraps
    the body in a For_i hardware loop (cheap to compile at any rep count)."""
    nc = bacc.Bacc("TRN2", target_bir_lowering=False, debug=False, num_devices=N_CORES)
    x = nc.dram_tensor("x", [BPC, HP, WP], BF16, kind="ExternalInput")
    out_kind = "Internal" if tiny_out else "ExternalOutput"
    out = nc.dram_tensor("out", [BPC, NR, 128, NBR, W], BF16, kind=out_kind)
    probe = (
        nc.dram_tensor("probe", [128, W], BF16, kind="ExternalOutput") if tiny_out else None
    )
    with tile.TileContext(nc) as tc:
        with (
            tc.tile_pool(name="tin", bufs=2) as tin,
            tc.tile_pool(name="tout", bufs=2) as tout,
        ):
            if loop:
                # staggered_reset: no all-engine barrier at the back edge —
                # iterations overlap like the unrolled-reps form does
                with tc.For_i(0, reps, 1, staggered_reset=True):
                    for b in range(BPC):
                        _build_image(nc, tin, tout, x, out, b)
            else:
                for _ in range(reps):
                    for b in range(BPC):
                        _build_image(nc, tin, tout, x, out, b)
            if probe is not None:
                pt = tin.tile([128, W], BF16, name="pt", tag="pt")
                nc.sync.dma_start(out=pt[:, :], in_=bass.AP(out, 0, [[W, 128], [1, W]]))
                nc.sync.dma_start(out=probe.ap(), in_=pt[:, :])
    nc.compile()
    return nc


def _get_nc():
    if "nc" not in _NC_CACHE:
        _NC_CACHE["nc"] = build()
    return _NC_CACHE["nc"]


def pad_input(x):
    """[16, 1, 512, 512] -> replicate-padded [16, 516, 516], bfloat16."""
    xs = np.asarray(x, dtype=np.float32).reshape(B_FULL, H, W)
    xp = np.pad(xs, ((0, 0), (PAD, PAD), (PAD, PAD)), mode="edge")
    return xp.astype(ml_dtypes.bfloat16)


def run(x, trace=False):
    nc = _get_nc()
    xp = pad_input(x)
    in_maps = [
        {"x": np.ascontiguousarray(xp[BPC * i : BPC * (i + 1)])} for i in range(N_CORES)
    ]
    res = run_bass_kernel_spmd(nc, in_maps, core_ids=list(range(N_CORES)), trace=trace)
    full = np.concatenate(
        [np.asarray(res.results[i]["out"]) for i in range(N_CORES)], axis=0
    )  # [16, NR(r), 128(p), 24, W] bf16; output row y = 4p + r
    out = (
        full.transpose(0, 3, 2, 1, 4).reshape(B_FULL, NBR, H, W).astype(np.float32)
    )
    return out, res


def kernel(x):
    return run(x)[0]
